# revision 38
# baseline (speedup 1.0000x reference)
"""LDPC belief-propagation kernel for Trainium2 (8 NeuronCores, data-parallel).

Tanh-product formulation (per row; H fixed [3,7], 12 edges, check-major
slots with each check's degree-1 "static" edge in slot 0):
  t_e   = tanh(m_e / 2)                       (signed; ACT Tanh)
  u_e   = prod_{e' in check(e), e'!=e} t_e'   (leave-one-out via pair trick)
  c2v_e = ln(1+u_e) - ln(1-u_e)               (= 2 artanh(u); sign comes free)
  new_llr_v = llr_v + sum_{c ni v} c2v_{c,v}
  m'_e  = new_llr_v - c2v_e
This needs only 3 ACT ops/iter (Tanh + 2 Ln) vs 8 for the log-domain form.
Leave-one-out uses pair products: P(pair) = t_a*t_b, then
u_e = t_partner(e) * P(other pair).  Degree-1 variables (v0,v1,v3) have
constant messages == llr: their t values are computed once ("static" slots
0,4,8); per-iteration work covers only the 9 dynamic edges.  Iteration 0
messages equal llr_v, so its tanh is folded into the one-time setup.

On-chip tiles are SLOT-MAJOR (slot*W + w) so every operand's innermost AP
dim is [1, W] — packed — which qualifies all fp16 DVE ops for the 2x_1p
fast mode regardless of slot strides.  Only the DMA-facing tiles (LLR in,
final new_llr out) stay w-major (7w+v) to keep transfers contiguous;
mixed-layout ops pair them with transposed, iteration-order-matched APs.

Engine split: all transcendentals on ACT; the c2v/new_llr/m' post-path on
DVE (fp16, 2x_1p; the last iteration writes the fp32 output tile).  The
t-products run on GPSIMD/Pool for three chunks and on DVE for one — Pool's
0.42 mult efficiency makes the optimal batch split uneven, and dedicating
one product engine per chunk keeps every instruction dependent on at most
one foreign engine (the sync-strip pass below requires a single wait slot
per instruction).

Slot layout (check-major):
  c0: [v0*, v2, v4, v6]  slots 0-3
  c1: [v1*, v2, v5, v6]  slots 4-7
  c2: [v3*, v4, v5, v6]  slots 8-11   (* = static, degree-1)
"""

import numpy as np

_CACHE = {}

NCORES = 8
P = 128                    # partitions
WS = (80, 74, 54, 48)      # free columns per partition per chunk (sum = Bc//P)
PRD_POOL = (True, False, True, True)  # product engine per chunk: Pool or DVE
WB = 3  # work-pool rotation depth
# One iteration's emission order: (stage, chunk, iteration-offset).  Stages of
# chunks with offset -1 are software-pipelined into the next iteration, which
# staggers the chunks' phases so no engine waits for the whole round trip.
# Safety: for each chunk, ("L", c) must precede ("T", c) of the next
# iteration in ACT order, and ("O", c, k) must precede ("P", c, k+1) on its
# product engine, else the single-wait FIFO queues deadlock.
SCHED_BLOCK = (
    ("T", 0, 0), ("P", 0, 0), ("L", 2, -1), ("O", 2, -1),
    ("T", 1, 0), ("P", 1, 0), ("L", 3, -1), ("O", 3, -1),
    ("T", 2, 0), ("P", 2, 0), ("L", 0, 0), ("O", 0, 0),
    ("T", 3, 0), ("P", 3, 0), ("L", 1, 0), ("O", 1, 0))

CA = 0.99999988  # Ln scale so the argument stays >= 6e-8 even at u == +-1
CB = 0.99999994  # keeps c2v finite and |c2v| <= ~16.8 (matches ref clamp)


def _build(Bc, iters):
    import contextlib

    import concourse.bass as bass
    import concourse.tile as tile
    from concourse import mybir
    from concourse.alu_op_type import AluOpType as Op

    F = mybir.ActivationFunctionType
    assert Bc == P * sum(WS), (Bc, WS)
    f32 = mybir.dt.float32
    f16 = mybir.dt.float16

    nc = bass.Bass("TRN2", target_bir_lowering=False, debug=False,
                   num_devices=1)
    llr_d = nc.dram_tensor("llr", [Bc, 7], f32, kind="ExternalInput")
    out_d = nc.dram_tensor("out", [Bc, 7], f32, kind="ExternalOutput")

    def sub(t, off, dims):
        a = t[:] if callable(getattr(t, "__getitem__", None)) else t
        return bass.AP(tensor=a.tensor, offset=a.offset + off,
                       ap=[list(a.ap[0])] + [list(d) for d in dims])

    def hbm_ap(t, row0, w):
        # [P, 7w] view of rows [row0, row0 + P*w): partition p <-> w rows
        a = t.ap()
        return bass.AP(tensor=a.tensor, offset=a.offset + 7 * row0,
                       ap=[[7 * w, P], [1, 7 * w]])

    with tile.TileContext(nc) as tc:
        ctx = contextlib.ExitStack()
        with ctx:
            keep = ctx.enter_context(tc.tile_pool(name="keep", bufs=1))
            work = ctx.enter_context(tc.tile_pool(name="work", bufs=WB))
            lastp = ctx.enter_context(tc.tile_pool(name="lastp", bufs=1))

            act = nc.scalar.activation
            vec = nc.vector
            gps = nc.gpsimd

            # Ln bias consts: one written by each product engine so the Ln's
            # bias-read dependency merges with its u-input wait (single
            # foreign-engine wait per instruction).
            CBBp = keep.tile([P, 1], f32, tag="CBBp", name="CBBp")
            gps.memset(CBBp, CB)
            CBBv = keep.tile([P, 1], f32, tag="CBBv", name="CBBv")
            vec.memset(CBBv, CB)
            CBBs = [CBBp if pp else CBBv for pp in PRD_POOL]

            def K(name, k, dt, w):
                return keep.tile([P, w * k], dt, tag=name, name=name)

            NCH = len(WS)
            LLRs = [K(f"LLR{c}", 7, f32, WS[c]) for c in range(NCH)]   # w-major
            LLVs = [K(f"LLV{c}", 7, f16, WS[c]) for c in range(NCH)]   # v-major
            Ts   = [K(f"T{c}", 12, f32, WS[c]) for c in range(NCH)]    # slot-major
            Ms   = [K(f"M{c}", 12, f16, WS[c]) for c in range(NCH)]    # slot-major

            def llr_tile_off(c):
                return (LLRs[c], 0)

            row0s = [P * sum(WS[:c]) for c in range(NCH)]
            for c in range(NCH):
                W, T = WS[c], Ts[c]
                LT, loff = llr_tile_off(c)
                nc.sync.dma_start(out=LT[:], in_=hbm_ap(llr_d, row0s[c], W))
                LLR = sub(LT, loff, [[1, 7 * W]])
                TL = lastp.tile([P, W * 7], f32, tag=f"TL{c}", name=f"TL{c}")
                act(TL[:], LLR, F.Tanh, scale=0.5)        # w-major
                # scatter tanh(llr/2) into the slot-major edge slots
                # (iteration-0 messages); in-APs iterate (slot, w) to match
                vec.tensor_copy(sub(T, 0, [[W, 4], [1, W]]),
                                sub(TL, 0, [[2, 4], [7, W]]))
                vec.tensor_copy(sub(T, 4 * W, [[W, 2], [1, W]]),
                                sub(TL, 1, [[1, 2], [7, W]]))
                vec.tensor_copy(sub(T, 6 * W, [[W, 2], [1, W]]),
                                sub(TL, 5, [[1, 2], [7, W]]))
                vec.tensor_copy(sub(T, 8 * W, [[W, 4], [1, W]]),
                                sub(TL, 3, [[1, 4], [7, W]]))
                # v-major fp16 llr copy for the packed mid-iteration NL adds
                vec.tensor_copy(sub(LLVs[c], 0, [[W, 7], [1, W]]),
                                sub(LT, loff, [[1, 7], [7, W]]))

            # --- stage emitters -------------------------------------------
            tiles = {}   # (kind, c, it) -> tile

            def dyn9(t, W):
                return sub(t, W, [[4 * W, 3], [W, 3], [1, W]])

            def g12(t, W):
                return sub(t, 0, [[W, 12], [1, W]])

            def em_tanh(c, it):
                if it == 0:
                    return   # iteration-0 t-values come from the setup scatter
                W = WS[c]
                act(dyn9(Ts[c], W), dyn9(Ms[c], W), F.Tanh, scale=0.5)

            def em_prods(c, it):
                W, T = WS[c], Ts[c]
                last = (it == iters - 1)
                prd = gps if PRD_POOL[c] else vec
                P6 = work.tile([P, W * 6], f32, tag=f"P6{c}", name=f"P6{c}")
                U = work.tile([P, W * 12], f32, tag=f"U{c}", name=f"U{c}")
                tiles[("U", c, it)] = U
                # pair products P6[2k+j] = t(4k+2j) * t(4k+2j+1)
                prd.tensor_tensor(sub(P6, 0, [[W, 6], [1, W]]),
                                  sub(T, 0, [[2 * W, 6], [1, W]]),
                                  sub(T, W, [[2 * W, 6], [1, W]]), Op.mult)
                # leave-one-out slots {2,3}: partner t * pair0 product
                prd.tensor_tensor(sub(U, 2 * W, [[4 * W, 3], [W, 2], [1, W]]),
                                  sub(T, 3 * W, [[4 * W, 3], [-W, 2], [1, W]]),
                                  sub(P6, 0, [[2 * W, 3], [0, 2], [1, W]]),
                                  Op.mult)
                # slot {1}: static-partner t * pair1 product
                prd.tensor_tensor(sub(U, W, [[4 * W, 3], [1, W]]),
                                  sub(T, 0, [[4 * W, 3], [1, W]]),
                                  sub(P6, W, [[2 * W, 3], [1, W]]), Op.mult)
                if last:
                    # static slots {0,4,8} (c2v for v0,v1,v3 outputs)
                    prd.tensor_tensor(sub(U, 0, [[4 * W, 3], [1, W]]),
                                      sub(T, W, [[4 * W, 3], [1, W]]),
                                      sub(P6, W, [[2 * W, 3], [1, W]]), Op.mult)

            def em_lns(c, it):
                W = WS[c]
                last = (it == iters - 1)
                U = tiles.pop(("U", c, it))
                A = work.tile([P, W * 12], f16, tag=f"Ah{c}", name=f"Ah{c}")
                B = work.tile([P, W * 12], f16, tag=f"Bh{c}", name=f"Bh{c}")
                tiles[("AB", c, it)] = (A, B)
                sl = (lambda t: g12(t, W)) if last else (lambda t: dyn9(t, W))
                act(sl(A), sl(U), F.Ln, bias=CBBs[c][:], scale=CA)
                act(sl(B), sl(U), F.Ln, bias=CBBs[c][:], scale=-CA)

            NLF23 = [None]

            def em_post(c, it):
                W, LLV, M = WS[c], LLVs[c], Ms[c]
                LT, loff = llr_tile_off(c)
                last = (it == iters - 1)
                A, B = tiles.pop(("AB", c, it))
                CV = A   # c2v computed in place over ln(1+u); slot-major f16
                sl = (lambda t: g12(t, W)) if last else (lambda t: dyn9(t, W))
                vec.tensor_tensor(sl(CV), sl(A), sl(B), Op.subtract)

                if last:
                    # fp32 w-major output tile; CV read via transposed APs
                    # whose iteration order (w, slot) matches the out AP.
                    # c2/c3 share one tile so a single DMA covers both.
                    NLT = lastp.tile([P, W * 7], f32, tag=f"NLf{c}",
                                     name=f"NLf{c}")
                    noff = 0
                    NL = sub(NLT, noff, [[1, 7 * W]])
                    LLR = sub(LT, loff, [[1, 7 * W]])
                    vec.tensor_tensor(sub(NL, 2, [[7, W], [2, 3]]),
                                      sub(LLR, 2, [[7, W], [2, 3]]),
                                      sub(CV, W, [[1, W], [W, 3]]), Op.add)
                    vec.tensor_tensor(sub(NL, 5, [[7, W], [1, 1]]),
                                      sub(LLR, 5, [[7, W], [1, 1]]),
                                      sub(CV, 6 * W, [[1, W], [0, 1]]), Op.add)
                    vec.tensor_tensor(sub(NL, 2, [[7, W], [4, 2]]),
                                      sub(NL, 2, [[7, W], [4, 2]]),
                                      sub(CV, 5 * W, [[1, W], [2 * W, 2]]),
                                      Op.add)
                    vec.tensor_tensor(sub(NL, 4, [[7, W], [1, 3]]),
                                      sub(NL, 4, [[7, W], [1, 3]]),
                                      sub(CV, 9 * W, [[1, W], [W, 3]]), Op.add)
                    vec.tensor_tensor(sub(NL, 0, [[7, W], [1, 2]]),
                                      sub(LLR, 0, [[7, W], [1, 2]]),
                                      sub(CV, 0, [[1, W], [4 * W, 2]]), Op.add)
                    vec.tensor_tensor(sub(NL, 3, [[7, W], [1, 1]]),
                                      sub(LLR, 3, [[7, W], [1, 1]]),
                                      sub(CV, 8 * W, [[1, W], [0, 1]]), Op.add)
                    nc.sync.dma_start(out=hbm_ap(out_d, row0s[c], W),
                                      in_=NLT[:])
                else:
                    # v-major fp16 new_llr; every operand's last dim is
                    # [1, W] so all ops run in the 2x_1p packed mode
                    NL = work.tile([P, W * 7], f16, tag=f"NLh{c}",
                                   name=f"NLh{c}")
                    vec.tensor_tensor(sub(NL, 2 * W, [[2 * W, 3], [1, W]]),
                                      sub(LLV, 2 * W, [[2 * W, 3], [1, W]]),
                                      sub(CV, W, [[W, 3], [1, W]]), Op.add)
                    vec.tensor_tensor(sub(NL, 5 * W, [[1, W]]),
                                      sub(LLV, 5 * W, [[1, W]]),
                                      sub(CV, 6 * W, [[1, W]]), Op.add)
                    vec.tensor_tensor(sub(NL, 2 * W, [[4 * W, 2], [1, W]]),
                                      sub(NL, 2 * W, [[4 * W, 2], [1, W]]),
                                      sub(CV, 5 * W, [[2 * W, 2], [1, W]]),
                                      Op.add)
                    vec.tensor_tensor(sub(NL, 4 * W, [[W, 3], [1, W]]),
                                      sub(NL, 4 * W, [[W, 3], [1, W]]),
                                      sub(CV, 9 * W, [[W, 3], [1, W]]), Op.add)
                    # m' = new_llr - c2v for the 9 dynamic edges
                    vec.tensor_tensor(sub(M, W, [[W, 3], [1, W]]),
                                      sub(NL, 2 * W, [[2 * W, 3], [1, W]]),
                                      sub(CV, W, [[W, 3], [1, W]]), Op.subtract)
                    vec.tensor_tensor(sub(M, 5 * W, [[1, W]]),
                                      sub(NL, 2 * W, [[1, W]]),
                                      sub(CV, 5 * W, [[1, W]]), Op.subtract)
                    vec.tensor_tensor(sub(M, 6 * W, [[W, 2], [1, W]]),
                                      sub(NL, 5 * W, [[W, 2], [1, W]]),
                                      sub(CV, 6 * W, [[W, 2], [1, W]]),
                                      Op.subtract)
                    vec.tensor_tensor(sub(M, 9 * W, [[W, 3], [1, W]]),
                                      sub(NL, 4 * W, [[W, 3], [1, W]]),
                                      sub(CV, 9 * W, [[W, 3], [1, W]]),
                                      Op.subtract)

            EMIT = {"T": em_tanh, "P": em_prods, "L": em_lns, "O": em_post}
            deferred = sorted((c for st, c, dk in SCHED_BLOCK
                               if dk and st == "L"), key=lambda c: -WS[c])
            for it in range(iters):
                for st, c, dk in SCHED_BLOCK:
                    k = it + dk
                    if 0 <= k < iters:
                        EMIT[st](c, k)
            for c in deferred:
                em_lns(c, iters - 1)
                em_post(c, iters - 1)

    # walrus on this stack supports a single sync-wait slot per instruction.
    # Tile emits (a) redundant same-engine waits (trivially satisfied by the
    # engine's FIFO program order once the preceding updates have happened)
    # and (b) a kernel-tail SP drain waiting on the whole global clock, where
    # only the output-DMA wait is load-bearing (the per-engine drain + EVSEM
    # butterfly that follows enforces engine completion).  Strip both.
    import bass_rust
    pref = {"EngineType.DVE": "DVE_", "EngineType.Pool": "Pool_",
            "EngineType.Activation": "Activation_", "EngineType.PE": "PE_",
            "EngineType.SP": "SP_"}
    inc = {}
    for b in nc.m.functions[0].blocks:
        for i in b.instructions:
            si = i.sync_info
            if si is None:
                continue
            if len(si.on_wait) > 1:
                if type(i).__name__ == "InstDrain":
                    dma = [w for w in si.on_wait if "DMA" in w.ant_name]
                    keep_w = dma[-1:] if dma else list(si.on_wait)[:1]
                else:
                    p = pref.get(str(i.engine))
                    keep_w = [w for w in si.on_wait
                              if not (p and w.ant_name.startswith(p)
                                      and w.wait_value <= inc.get(w.ant_name, 0))]
                    assert len(keep_w) <= 1, (i.name, [(w.ant_name, w.wait_value) for w in keep_w], {k: inc.get(k) for k in [w.ant_name for w in si.on_wait]})
                i.sync_info = bass_rust.SyncInfo(on_wait=keep_w,
                                                on_update=list(si.on_update))
                si = i.sync_info
            for u in si.on_update:
                if u.update_mode == "sem-inc":
                    inc[u.ant_name] = inc.get(u.ant_name, 0) + u.update_value
    return nc


def kernel(llr, max_iters):
    llr = np.ascontiguousarray(np.asarray(llr), dtype=np.float32)
    iters = int(np.asarray(max_iters))
    B = llr.shape[0]
    if iters <= 0:
        return llr.reshape(B, 1, 7).copy()

    from concourse.bass_utils import run_bass_kernel_spmd

    Bc = B // NCORES
    key = (Bc, iters)
    if key not in _CACHE:
        _CACHE[key] = _build(Bc, iters)
    nc = _CACHE[key]

    flat = llr.reshape(B, 7)
    in_maps = [{"llr": flat[i * Bc:(i + 1) * Bc]} for i in range(NCORES)]
    res = run_bass_kernel_spmd(nc, in_maps, core_ids=list(range(NCORES)))
    out = np.concatenate([np.asarray(r["out"]) for r in res.results], axis=0)
    return out.reshape(B, 1, 7)
